# revision 1
# baseline (speedup 1.0000x reference)
"""HAFS hierarchy-aware loss kernel for 8 TRN2 NeuronCores.

Strategy (sharding_hint: shard projections / leaf axis of `out`, replicate
logits/basis, combine scalar loss terms):
  - 250 leaf classes padded to 256, 32 leaves per core.
  - Host pre-transposes projections so the PE contracts over d directly:
    per leaf G[b, c] = sum_d logits[b, d] * proj[c, d] accumulates in PSUM
    from 4 K-chunks (128+128+128+51); the ScalarE squares + row-reduces the
    PSUM tile, giving ||P_n x_b||^2; sqrt at the end.
  - The small loss terms (basis projection alignment + KL vs label mass) are
    computed redundantly on every core (few us, rides under the DMA-bound
    einsum); host combines core 0's partial terms into the final scalar.
  - Variant "hilo": projections and logits shipped as bf16 hi/lo pairs and
    the einsum runs 3 bf16 passes (Ph*xh + Pl*xh + Ph*xl). Same HBM bytes as
    fp32 but 3 cyc/row on the PE instead of fp32's 4, with ~1e-5 rel error.
"""

import os
import sys

sys.path.insert(0, "/opt/trn_rl_repo")

import numpy as np
import ml_dtypes

LEAF = 250
NODES = [10, 50, 125, 250]
OFFS = [0, 10, 60, 185]
DIM = 435
BATCH = 64
ALPHA = 0.1
N_CORES = 8
LPC = 32          # leaves per core (250 padded to 256)
GL = 4            # leaves per DMA group
NG = LPC // GL    # groups per core
P = 128
KCH = 3           # full 128-row K chunks
DT_MAIN = KCH * P # 384
DT_TAIL = DIM - DT_MAIN  # 51
NTERMS = 10

VARIANT = os.environ.get("HAFS_VARIANT", "hilo")  # fp32 | fp32r | bf16 | hilo

_prog_cache = {}
LAST_RESULTS = None

BF16 = ml_dtypes.bfloat16


def _split_hilo(x):
    hi = x.astype(BF16)
    lo = (x - hi.astype(np.float32)).astype(BF16)
    return hi, lo


def _build_program(variant):
    import concourse.bass as bass  # noqa: F401
    import concourse.tile as tile
    from concourse import bacc, mybir

    f32 = mybir.dt.float32
    if variant in ("fp32",):
        pdt = mybir.dt.float32
    elif variant == "fp32r":
        pdt = mybir.dt.float32r
    else:
        pdt = mybir.dt.bfloat16
    hilo = variant == "hilo"

    AF = mybir.ActivationFunctionType
    ALU = mybir.AluOpType
    AX = mybir.AxisListType

    nc = bacc.Bacc("TRN2", target_bir_lowering=False, debug=False,
                   num_devices=N_CORES)

    # ---- DRAM I/O ----
    def din(name, shape, dt=f32):
        return nc.dram_tensor(name, list(shape), dt, kind="ExternalInput").ap()

    def dout(name, shape, dt=f32):
        return nc.dram_tensor(name, list(shape), dt, kind="ExternalOutput").ap()

    if hilo:
        pbh = din("pbh", [NG, P, GL, KCH, DIM], pdt)
        pbl = din("pbl", [NG, P, GL, KCH, DIM], pdt)
        pth = din("pth", [DT_TAIL, NG, GL, DIM], pdt)
        ptl = din("ptl", [DT_TAIL, NG, GL, DIM], pdt)
        xmh = din("xmh", [P, KCH, BATCH], pdt)
        xml = din("xml", [P, KCH, BATCH], pdt)
        xth = din("xth", [DT_TAIL, BATCH], pdt)
        xtl = din("xtl", [DT_TAIL, BATCH], pdt)
    else:
        pb = din("pb", [NG, P, GL, KCH, DIM], pdt)
        pt = din("pt", [DT_TAIL, NG, GL, DIM], pdt)
        xm = din("xm", [P, KCH, BATCH], pdt)
        xt = din("xt", [DT_TAIL, BATCH], pdt)

    lg = din("lg", [BATCH, DIM])           # logits, natural layout
    bm = din("bm", [P, KCH, DIM])          # basis.T main chunks
    bt = din("bt", [DT_TAIL, DIM])         # basis.T tail chunk
    x32m = din("x32m", [P, KCH, BATCH])    # logits.T fp32 (for basis matmul)
    x32t = din("x32t", [DT_TAIL, BATCH])
    ohe = din("ohe", [BATCH, DIM])         # per-level one-hots, concatenated
    lgt = din("lgt", [BATCH, DIM])         # l_gt
    pw = din("pw", [BATCH, 4])             # posf per level
    nw = din("nw", [BATCH, 4])             # negf per level
    rs = din("rs", [BATCH, 1])             # row-sums of l_gt
    on_ = din("on", [BATCH, 1])            # ones (partition-reduce weight)

    o_out = dout("o", [BATCH, LPC])
    t_out = dout("t", [1, NTERMS])

    with tile.TileContext(nc) as tc, \
         tc.tile_pool(name="const", bufs=1) as constp, \
         tc.tile_pool(name="proj", bufs=3) as projp, \
         tc.tile_pool(name="tail", bufs=3) as tailp, \
         tc.tile_pool(name="scr", bufs=4) as scrp, \
         tc.tile_pool(name="loss", bufs=1) as lossp, \
         tc.tile_pool(name="psA", bufs=6, space="PSUM") as psA, \
         tc.tile_pool(name="psB", bufs=1, space="PSUM") as psB, \
         tc.tile_pool(name="psR", bufs=1, space="PSUM") as psR:

        # ---- load constants ----
        if hilo:
            xmh_s = constp.tile([P, KCH, BATCH], pdt)
            nc.sync.dma_start(xmh_s[:], xmh)
            xml_s = constp.tile([P, KCH, BATCH], pdt)
            nc.sync.dma_start(xml_s[:], xml)
            xth_s = constp.tile([DT_TAIL, BATCH], pdt)
            nc.sync.dma_start(xth_s[:], xth)
            xtl_s = constp.tile([DT_TAIL, BATCH], pdt)
            nc.sync.dma_start(xtl_s[:], xtl)
        else:
            xm_s = constp.tile([P, KCH, BATCH], pdt)
            nc.sync.dma_start(xm_s[:], xm)
            xt_s = constp.tile([DT_TAIL, BATCH], pdt)
            nc.sync.dma_start(xt_s[:], xt)

        x32m_s = constp.tile([P, KCH, BATCH], f32)
        nc.sync.dma_start(x32m_s[:], x32m)
        x32t_s = constp.tile([DT_TAIL, BATCH], f32)
        nc.sync.dma_start(x32t_s[:], x32t)
        bm_s = constp.tile([P, KCH, DIM], f32)
        nc.sync.dma_start(bm_s[:], bm)
        bt_s = constp.tile([DT_TAIL, DIM], f32)
        nc.sync.dma_start(bt_s[:], bt)
        lg_s = constp.tile([BATCH, DIM], f32)
        nc.sync.dma_start(lg_s[:], lg)
        ohe_s = constp.tile([BATCH, DIM], f32)
        nc.sync.dma_start(ohe_s[:], ohe)
        lgt_s = constp.tile([BATCH, DIM], f32)
        nc.sync.dma_start(lgt_s[:], lgt)
        pw_s = constp.tile([BATCH, 4], f32)
        nc.sync.dma_start(pw_s[:], pw)
        nw_s = constp.tile([BATCH, 4], f32)
        nc.sync.dma_start(nw_s[:], nw)
        rs_s = constp.tile([BATCH, 1], f32)
        nc.sync.dma_start(rs_s[:], rs)
        on_s = constp.tile([BATCH, 1], f32)
        nc.sync.dma_start(on_s[:], on_)

        sums = constp.tile([BATCH, LPC], f32)  # per-leaf sum of squares

        # ---- main einsum over this core's 32 leaves ----
        for g in range(NG):
            if hilo:
                bh = projp.tile([P, GL, KCH, DIM], pdt, tag="bh")
                nc.sync.dma_start(bh[:], pbh[g])
                bl = projp.tile([P, GL, KCH, DIM], pdt, tag="bl")
                nc.sync.dma_start(bl[:], pbl[g])
                th_ = tailp.tile([DT_TAIL, GL, DIM], pdt, tag="th")
                nc.sync.dma_start(th_[:], pth[:, g])
                tl_ = tailp.tile([DT_TAIL, GL, DIM], pdt, tag="tl")
                nc.sync.dma_start(tl_[:], ptl[:, g])
            else:
                bg = projp.tile([P, GL, KCH, DIM], pdt, tag="bg")
                nc.sync.dma_start(bg[:], pb[g])
                tg = tailp.tile([DT_TAIL, GL, DIM], pdt, tag="tg")
                nc.sync.dma_start(tg[:], pt[:, g])

            for l in range(GL):
                ps = psA.tile([BATCH, DIM], f32, tag="ps")
                if hilo:
                    mms = []
                    for k in range(KCH):
                        mms += [(xmh_s[:, k], bh[:, l, k]),
                                (xml_s[:, k], bh[:, l, k]),
                                (xmh_s[:, k], bl[:, l, k])]
                    mms += [(xth_s, th_[:, l]),
                            (xtl_s, th_[:, l]),
                            (xth_s, tl_[:, l])]
                else:
                    mms = [(xm_s[:, k], bg[:, l, k]) for k in range(KCH)]
                    mms.append((xt_s, tg[:, l]))
                for i, (w, r) in enumerate(mms):
                    nc.tensor.matmul(ps[:], w, r, start=(i == 0),
                                     stop=(i == len(mms) - 1))
                j = g * GL + l
                scr = scrp.tile([BATCH, DIM], f32, tag="scr")
                nc.scalar.activation(scr[:], ps[:], AF.Square,
                                     accum_out=sums[:, j:j + 1])

        out_sb = constp.tile([BATCH, LPC], f32)
        nc.scalar.activation(out_sb[:], sums[:], AF.Sqrt)
        nc.sync.dma_start(o_out, out_sb[:])

        # ---- loss terms (replicated on every core) ----
        # B = logits @ basis.T  ->  LP = |B|
        bps = psB.tile([BATCH, DIM], f32)
        for k in range(KCH):
            nc.tensor.matmul(bps[:], x32m_s[:, k], bm_s[:, k],
                             start=(k == 0), stop=False)
        nc.tensor.matmul(bps[:], x32t_s[:], bt_s[:], start=False, stop=True)
        LP = lossp.tile([BATCH, DIM], f32)
        nc.scalar.activation(LP[:], bps[:], AF.Abs)

        nrm2 = lossp.tile([BATCH, 4], f32)
        for li, (o0, n_l) in enumerate(zip(OFFS, NODES)):
            s1 = scrp.tile([BATCH, DIM], f32, tag="scr")
            nc.scalar.activation(s1[:, :n_l], LP[:, o0:o0 + n_l], AF.Square,
                                 accum_out=nrm2[:, li:li + 1])
        nrm = lossp.tile([BATCH, 4], f32)
        nc.scalar.activation(nrm[:], nrm2[:], AF.Sqrt)
        rcp = lossp.tile([BATCH, 4], f32)
        nc.vector.reciprocal(rcp[:], nrm[:])

        T = lossp.tile([BATCH, 4], f32)    # sum_c |ohe - cos| per level
        NLs = lossp.tile([BATCH, 4], f32)  # sum_c LP per level
        for li, (o0, n_l) in enumerate(zip(OFFS, NODES)):
            t1 = scrp.tile([BATCH, DIM], f32, tag="scr")
            nc.vector.tensor_scalar_mul(t1[:, :n_l], LP[:, o0:o0 + n_l],
                                        rcp[:, li:li + 1])
            nc.vector.tensor_sub(t1[:, :n_l], t1[:, :n_l],
                                 ohe_s[:, o0:o0 + n_l])
            s2 = scrp.tile([BATCH, DIM], f32, tag="scr")
            nc.scalar.activation(s2[:, :n_l], t1[:, :n_l], AF.Abs,
                                 accum_out=T[:, li:li + 1])
            nc.vector.tensor_reduce(NLs[:, li:li + 1], LP[:, o0:o0 + n_l],
                                    axis=AX.X, op=ALU.add)

        V = lossp.tile([BATCH, NTERMS], f32)
        nc.vector.tensor_mul(V[:, 0:4], T[:], pw_s[:])
        nc.vector.tensor_mul(V[:, 4:8], NLs[:], nw_s[:])

        # log-softmax pieces over |logits|
        A = lossp.tile([BATCH, DIM], f32)
        nc.scalar.activation(A[:], lg_s[:], AF.Abs)
        mx = lossp.tile([BATCH, 1], f32)
        nc.vector.tensor_reduce(mx[:], A[:], axis=AX.X, op=ALU.max)
        ngm = lossp.tile([BATCH, 1], f32)
        nc.vector.tensor_scalar_mul(ngm[:], mx[:], -1.0)
        se = scrp.tile([BATCH, DIM], f32, tag="scr")
        ssum = lossp.tile([BATCH, 1], f32)
        nc.scalar.activation(se[:], A[:], AF.Exp, bias=ngm[:],
                             accum_out=ssum[:])
        lgs = lossp.tile([BATCH, 1], f32)
        nc.scalar.activation(lgs[:], ssum[:], AF.Ln)
        mls = lossp.tile([BATCH, 1], f32)
        nc.vector.tensor_add(mls[:], mx[:], lgs[:])
        nc.vector.tensor_mul(V[:, 9:10], rs_s[:], mls[:])

        sd = scrp.tile([BATCH, DIM], f32, tag="scr")
        nc.vector.tensor_mul(sd[:], lgt_s[:], A[:])
        nc.vector.tensor_reduce(V[:, 8:9], sd[:], axis=AX.X, op=ALU.add)

        red = psR.tile([1, NTERMS], f32)
        nc.tensor.matmul(red[:], on_s[:], V[:], start=True, stop=True)
        tsb = lossp.tile([1, NTERMS], f32)
        nc.vector.tensor_copy(tsb[:], red[:])
        nc.sync.dma_start(t_out, tsb[:])

    nc.compile()
    return nc


def _host_prep(logits, projections, basis, node_prob, labels, hier_labels,
               variant):
    """Build the per-core input maps + host-side scalars."""
    f32 = np.float32
    logits = np.ascontiguousarray(logits, f32)
    basis = np.ascontiguousarray(basis, f32)
    node_prob = np.asarray(node_prob, f32)
    labels = np.asarray(labels).astype(np.int64)
    hier = np.asarray(hier_labels).astype(np.int64)

    # --- projections -> per-core SBUF-ready layout ---
    nl = projections.shape[0]
    pad = N_CORES * LPC - nl
    proj = np.concatenate(
        [np.asarray(projections, f32),
         np.zeros((pad,) + projections.shape[1:], f32)], axis=0)
    # main [core, g, p, l, k, c]  (d = k*128 + p)
    main = proj[:, :, :DT_MAIN].reshape(N_CORES, NG, GL, DIM, KCH, P)
    main = np.ascontiguousarray(main.transpose(0, 1, 5, 2, 4, 3))
    main = main.reshape(N_CORES, NG, P, GL, KCH, DIM)
    # tail [core, p(51), g, l, c]  (d = 384 + p)
    tailv = proj[:, :, DT_MAIN:].reshape(N_CORES, NG, GL, DIM, DT_TAIL)
    tailv = np.ascontiguousarray(tailv.transpose(0, 4, 1, 2, 3))
    tailv = tailv.reshape(N_CORES, DT_TAIL, NG, GL, DIM)

    # --- logits transposed chunks ---
    lt = np.ascontiguousarray(logits.T)                       # [435, 64]
    x32m = np.ascontiguousarray(
        lt[:DT_MAIN].reshape(KCH, P, BATCH).transpose(1, 0, 2))
    x32t = np.ascontiguousarray(lt[DT_MAIN:])

    # --- basis.T chunks ---
    bT = np.ascontiguousarray(basis.T)                        # [435, 435]
    bm = np.ascontiguousarray(
        bT[:DT_MAIN].reshape(KCH, P, DIM).transpose(1, 0, 2))
    bt = np.ascontiguousarray(bT[DT_MAIN:])

    # --- label-derived small tensors (host) ---
    lvl = hier[labels]                                        # [64, 4]
    posf = (lvl >= 0).astype(f32)                             # [64, 4]
    negf = (1.0 - posf).astype(f32)
    n_pos = posf.sum(axis=0)
    n_neg = negf.sum(axis=0)
    gpos = np.array([float(np.any(lvl[:, li] > 0)) for li in range(4)], f32)
    gneg = (n_neg > 0).astype(f32)

    ohe = np.zeros((BATCH, DIM), f32)
    lgt = np.zeros((BATCH, DIM), f32)
    for li, (o0, n_l) in enumerate(zip(OFFS, NODES)):
        cl = np.clip(lvl[:, li], 0, None)
        oh = (cl[:, None] == np.arange(n_l)[None, :]).astype(f32)
        ohe[:, o0:o0 + n_l] = oh
        lgt[:, o0:o0 + n_l] = oh * (node_prob[labels, li] * posf[:, li])[:, None]
    rsum = lgt.sum(axis=1, keepdims=True).astype(f32)
    with np.errstate(divide="ignore", invalid="ignore"):
        xlogy = np.where(lgt > 0, lgt * np.log(np.where(lgt > 0, lgt, 1.0)), 0.0)
    xlogy_sum = float(xlogy.astype(np.float64).sum())

    common = {
        "lg": logits,
        "bm": bm, "bt": bt, "x32m": x32m, "x32t": x32t,
        "ohe": ohe, "lgt": lgt,
        "pw": np.ascontiguousarray(posf), "nw": np.ascontiguousarray(negf),
        "rs": rsum, "on": np.ones((BATCH, 1), f32),
    }

    if variant == "hilo":
        xh, xl = _split_hilo(lt[:DT_MAIN].reshape(KCH, P, BATCH)
                             .transpose(1, 0, 2).copy())
        xth_, xtl_ = _split_hilo(lt[DT_MAIN:].copy())
        mh, ml = _split_hilo(main)
        th_, tl_ = _split_hilo(tailv)
        per_core = [
            dict(common, pbh=mh[c], pbl=ml[c], pth=th_[c], ptl=tl_[c],
                 xmh=xh, xml=xl, xth=xth_, xtl=xtl_)
            for c in range(N_CORES)
        ]
    else:
        cast = (lambda a: a) if variant in ("fp32", "fp32r") \
            else (lambda a: a.astype(BF16))
        per_core = [
            dict(common, pb=cast(main[c]), pt=cast(tailv[c]),
                 xm=cast(x32m), xt=cast(x32t))
            for c in range(N_CORES)
        ]

    host = dict(gpos=gpos, gneg=gneg, n_pos=n_pos, n_neg=n_neg,
                xlogy_sum=xlogy_sum)
    return per_core, host


def _combine_loss(terms, host):
    """terms: [10] = [T0..T3 (posf-weighted), NL0..NL3 (negf-weighted),
    sum(lgt*|logits|), sum(rsum*(max+log(sumexp)))]"""
    l_aux = 0.0
    for li in range(4):
        pos_term = terms[li] / max(host["n_pos"][li], 1.0)
        neg_term = terms[4 + li] / max(host["n_neg"][li], 1.0)
        l_aux += host["gpos"][li] * pos_term + host["gneg"][li] * neg_term
    l_kl = (host["xlogy_sum"] - terms[8] + terms[9]) / BATCH
    return np.float32(l_kl + ALPHA * l_aux)


def kernel(**inputs):
    global LAST_RESULTS
    variant = VARIANT
    per_core, host = _host_prep(
        inputs["logits"], inputs["projections"], inputs["basis"],
        inputs["node_prob"], inputs["labels"], inputs["hier_labels"], variant)

    if variant not in _prog_cache:
        _prog_cache[variant] = _build_program(variant)
    nc = _prog_cache[variant]

    from concourse.bass_utils import run_bass_kernel_spmd
    res = run_bass_kernel_spmd(nc, per_core, core_ids=list(range(N_CORES)))
    LAST_RESULTS = res

    out = np.zeros((BATCH, LEAF), np.float32)
    for c in range(N_CORES):
        lo = c * LPC
        hi = min(lo + LPC, LEAF)
        out[:, lo:hi] = res.results[c]["o"][:, :hi - lo]
    loss = _combine_loss(np.asarray(res.results[0]["t"], np.float64)[0], host)
    return loss, out
